# revision 10
# baseline (speedup 1.0000x reference)
"""BatchLSTMCRFTagger Trainium2 Bass kernel.

Sharding: data-parallel over batch (B=64) across 8 NeuronCores, 8 examples
per core; LSTM/CRF params replicated. Each core runs the full 2-layer BiLSTM
+ output projection + CRF Viterbi forward for its slice; the (integer) path
backtrace is assembled on host from device-computed backpointers.

Per-core pipeline (fp16 matmuls, fp32 accumulation/state, fp32 CRF):
  embedding gather (indirect DMA) -> PE transpose -> xT [E,(t,b)] fp16
  input projections as large batched matmuls -> xp scratch (DRAM, fp32)
  LSTM scans: stationary W_hh.T tiles [128h,128g] on PE, moving h [128h,8b];
    fwd+rev chains of each layer interleaved to hide the per-step
    elementwise latency
  feats = W_out @ concat(h1f,h1r) on PE; CRF Viterbi on DVE (32x32
    stream-transpose + max8/max_index per step).
"""
import os
import numpy as np
from contextlib import ExitStack

V, E, H, L, T, B, K = 50000, 256, 512, 2, 256, 64, 32
NCORES = 8
BC = B // NCORES  # 8 examples per core

_T = int(os.environ.get("KERNEL_T", str(T)))


def _f16(x):
    return np.ascontiguousarray(np.asarray(x, np.float32).astype(np.float16))


def _gate_reorder_index():
    # pytorch gate rows [i(0:H) | f(H:2H) | g(2H:3H) | o(3H:4H)] -> chunk
    # m = 4*j + {0:i, 1:f, 2:o, 3:g} for h-chunk j (128 units each)
    idx = np.zeros(4 * H, np.int64)
    pos = 0
    for j in range(4):
        for base in (0, H, 3 * H, 2 * H):  # i, f, o, g
            idx[pos:pos + 128] = np.arange(base + 128 * j, base + 128 * (j + 1))
            pos += 128
    return idx


_GIDX = _gate_reorder_index()


def _prep_wT(w):
    # [4H, D] -> stationary tiles [128(p=contraction within chunk), D//128(k),
    #                              16(m), 128(col=gate)] fp16
    w = np.asarray(w, np.float32)[_GIDX]
    D = w.shape[1]
    wT = w.T.reshape(D // 128, 128, 16, 128)
    return _f16(wT.transpose(1, 0, 2, 3))


def build_kernel():
    import concourse.bass as bass
    import concourse.tile as tile
    from concourse import bacc, mybir
    from concourse.masks import make_identity

    F16 = mybir.dt.float16
    F32 = mybir.dt.float32
    I32 = mybir.dt.int32
    U32 = mybir.dt.uint32
    AF = mybir.ActivationFunctionType
    OP = mybir.AluOpType

    Tn = _T
    NB = Tn * BC
    BLK = min(512, NB)
    NBLK = NB // BLK
    TPB = BLK // BC  # timesteps per block

    nc = bacc.Bacc("TRN2", target_bir_lowering=False, debug=False)

    emb_in = nc.dram_tensor("emb", [V, E], F16, kind="ExternalInput").ap()
    toks_in = nc.dram_tensor("toks", [Tn * BC, 1], I32, kind="ExternalInput").ap()
    whh_in = nc.dram_tensor("whh", [4, 128, 4, 16, 128], F16, kind="ExternalInput").ap()
    wih0_in = nc.dram_tensor("wih0", [2, 128, 2, 16, 128], F16, kind="ExternalInput").ap()
    wih1_in = nc.dram_tensor("wih1", [2, 128, 8, 16, 128], F16, kind="ExternalInput").ap()
    bias_in = nc.dram_tensor("bias", [128, 4, 16], F32, kind="ExternalInput").ap()
    woutT_in = nc.dram_tensor("woutT", [128, 8, 32], F16, kind="ExternalInput").ap()
    transT_in = nc.dram_tensor("transT", [128, K], F32, kind="ExternalInput").ap()
    start_in = nc.dram_tensor("startc", [128, 1], F32, kind="ExternalInput").ap()
    stop_in = nc.dram_tensor("stopc", [128, 1], F32, kind="ExternalInput").ap()

    scores_out = nc.dram_tensor("scores", [2, 128, 8], F32, kind="ExternalOutput").ap()
    last_out = nc.dram_tensor("last", [2, 128, 8], U32, kind="ExternalOutput").ap()
    bp_out = nc.dram_tensor("bp", [2, 128, Tn, 8], U32, kind="ExternalOutput").ap()

    with tile.TileContext(nc) as tc, ExitStack() as ctx:
        const = ctx.enter_context(tc.tile_pool(name="const", bufs=1))
        wpool = ctx.enter_context(tc.tile_pool(name="wpool", bufs=2))
        temps = ctx.enter_context(tc.tile_pool(name="temps", bufs=3))
        xppool = ctx.enter_context(tc.tile_pool(name="xppool", bufs=4))
        srcpool = ctx.enter_context(tc.tile_pool(name="srcpool", bufs=10))
        psum = ctx.enter_context(tc.tile_pool(name="psum", bufs=2, space="PSUM"))
        psum_x = ctx.enter_context(tc.tile_pool(name="psum_x", bufs=2, space="PSUM"))
        dram = ctx.enter_context(tc.tile_pool(name="dram", bufs=1, space="DRAM"))

        xp_t, h_t = [], []
        for l in range(2):
            xpl = dram.tile([2, Tn, 128, 4, 4, BC], F32, tag=f"xp{l}", name=f"xp{l}")
            xp_t.append(xpl)
            hl = dram.tile([2, 4, 128, Tn, BC], F16, tag=f"h{l}", name=f"h{l}")
            h_t.append(hl)

        ident = const.tile([128, 128], F16)
        make_identity(nc, ident[:])

        # ---- embedding gather + transpose -> xT [128, 2, NB] fp16 ----
        toks_sb = const.tile([128, NB // 128], I32)
        nc.sync.dma_start(toks_sb[:], toks_in.rearrange("(a p) o -> p (a o)", p=128))
        xT = const.tile([128, 2, NB], F16)
        for i in range(NB // 128):
            row = temps.tile([128, 256], F16, tag="erow")
            nc.gpsimd.indirect_dma_start(
                out=row[:], out_offset=None, in_=emb_in[:],
                in_offset=bass.IndirectOffsetOnAxis(ap=toks_sb[:, i:i + 1], axis=0),
            )
            for c in range(2):
                pt = psum_x.tile([128, 128], F16, tag="px")
                nc.tensor.transpose(pt[:], row[:, c * 128:(c + 1) * 128], ident[:])
                nc.vector.tensor_copy(xT[:, c, i * 128:(i + 1) * 128], pt[:])

        bias_sb = const.tile([128, 4, 16], F32)
        nc.sync.dma_start(bias_sb[:], bias_in[:])

        def xp_project(layer, wih_tiles, kchunks, get_srcs):
            for d in range(2):
                for nb in range(NBLK):
                    srcs = get_srcs(d, nb)
                    for m in range(16):
                        ps = psum_x.tile([128, BLK], F32, tag="px")
                        for k in range(kchunks):
                            nc.tensor.matmul(
                                ps[:], lhsT=wih_tiles[d][:, k, m, :], rhs=srcs[k],
                                start=(k == 0), stop=(k == kchunks - 1),
                            )
                        xs = temps.tile([128, BLK], F32, tag="xpadd")
                        nc.vector.tensor_tensor(
                            out=xs[:], in0=ps[:],
                            in1=bias_sb[:, 2 * layer + d, m:m + 1].to_broadcast((128, BLK)),
                            op=OP.add,
                        )
                        nc.sync.dma_start(
                            xp_t[layer][d, nb * TPB:(nb + 1) * TPB, :, m // 4, m % 4, :]
                            .rearrange("t p b -> p t b"),
                            xs[:].rearrange("p (t b) -> p t b", b=BC),
                        )

        def lstm_scan(layer):
            h_st = [const.tile([128, 4, BC], F16, tag=f"h{layer}{d}", name=f"h{layer}{d}")
                    for d in range(2)]
            c_st = [const.tile([128, 4, BC], F32, tag=f"c{layer}{d}", name=f"c{layer}{d}")
                    for d in range(2)]
            for d in range(2):
                nc.vector.memset(h_st[d][:], 0.0)
                nc.vector.memset(c_st[d][:], 0.0)
            for t in range(Tn):
                for d in range(2):
                    tt = t if d == 0 else Tn - 1 - t
                    xpt = xppool.tile([128, 4, 4, BC], F32, tag=f"xpt{d}")
                    nc.sync.dma_start(xpt[:], xp_t[layer][d, tt])
                    ps = psum.tile([128, 4, 4, BC], F32, tag=f"ps{d}")
                    for m in range(16):
                        for k in range(4):
                            nc.tensor.matmul(
                                ps[:, m // 4, m % 4, :],
                                lhsT=whh_sb[d][:, k, m, :],
                                rhs=h_st[d][:, k, :],
                                start=(k == 0), stop=(k == 3),
                            )
                    g_sb = temps.tile([128, 4, 4, BC], F32, tag=f"g{d}")
                    nc.vector.tensor_tensor(out=g_sb[:], in0=ps[:], in1=xpt[:], op=OP.add)
                    s_sb = temps.tile([128, 4, 3, BC], F32, tag=f"s{d}")
                    nc.scalar.activation(s_sb[:], g_sb[:, :, 0:3, :], AF.Sigmoid)
                    tg_sb = temps.tile([128, 4, BC], F32, tag=f"tg{d}")
                    nc.scalar.activation(tg_sb[:], g_sb[:, :, 3, :], AF.Tanh)
                    t1 = temps.tile([128, 4, BC], F32, tag=f"t1{d}")
                    nc.vector.tensor_tensor(out=t1[:], in0=s_sb[:, :, 0, :], in1=tg_sb[:], op=OP.mult)
                    t2 = temps.tile([128, 4, BC], F32, tag=f"t2{d}")
                    nc.vector.tensor_tensor(out=t2[:], in0=s_sb[:, :, 1, :], in1=c_st[d][:], op=OP.mult)
                    nc.vector.tensor_tensor(out=c_st[d][:], in0=t1[:], in1=t2[:], op=OP.add)
                    th = temps.tile([128, 4, BC], F32, tag=f"th{d}")
                    nc.scalar.activation(th[:], c_st[d][:], AF.Tanh)
                    nc.vector.tensor_tensor(out=h_st[d][:], in0=s_sb[:, :, 2, :], in1=th[:], op=OP.mult)
                    nc.sync.dma_start(
                        h_t[layer][d, :, :, tt, :].rearrange("k p b -> p k b"),
                        h_st[d][:],
                    )

        # ===================== layer 0 =====================
        wih0_sb = []
        for d in range(2):
            w = wpool.tile([128, 2, 16, 128], F16, tag="w")
            nc.sync.dma_start(w[:], wih0_in[d])
            wih0_sb.append(w)
        xp_project(0, wih0_sb, 2,
                   lambda d, nb: [xT[:, k, nb * BLK:(nb + 1) * BLK] for k in range(2)])

        whh_sb = []
        for d in range(2):
            w = wpool.tile([128, 4, 16, 128], F16, tag="w")
            nc.sync.dma_start(w[:], whh_in[0 * 2 + d])
            whh_sb.append(w)
        lstm_scan(0)

        # ===================== layer 1 =====================
        def h_srcs(htile):
            def get(d, nb):
                out = []
                for k in range(8):
                    tl = srcpool.tile([128, BLK], F16, tag="src")
                    nc.sync.dma_start(
                        tl[:],
                        htile[k // 4, k % 4, :, nb * TPB:(nb + 1) * TPB, :]
                        .rearrange("p t b -> p (t b)"),
                    )
                    out.append(tl[:])
                return out
            return get

        wih1_sb = []
        for d in range(2):
            w = wpool.tile([128, 8, 16, 128], F16, tag="w")
            nc.sync.dma_start(w[:], wih1_in[d])
            wih1_sb.append(w)
        xp_project(1, wih1_sb, 8, h_srcs(h_t[0]))

        whh_sb = []
        for d in range(2):
            w = wpool.tile([128, 4, 16, 128], F16, tag="w")
            nc.sync.dma_start(w[:], whh_in[1 * 2 + d])
            whh_sb.append(w)
        lstm_scan(1)

        # ============ feats + CRF ============
        wout_sb = const.tile([128, 8, 32], F16)
        nc.sync.dma_start(wout_sb[:], woutT_in[:])
        feats_sb = const.tile([32, Tn, BC], F32)
        get1 = h_srcs(h_t[1])
        for nb in range(NBLK):
            srcs = get1(0, nb)
            fps = psum_x.tile([32, BLK], F32, tag="px")
            for k in range(8):
                nc.tensor.matmul(fps[:], lhsT=wout_sb[:, k, :], rhs=srcs[k],
                                 start=(k == 0), stop=(k == 7))
            nc.vector.tensor_copy(
                feats_sb[:, nb * TPB:(nb + 1) * TPB, :],
                fps[:].rearrange("p (t b) -> p t b", b=BC),
            )

        transT_sb = const.tile([128, K], F32)
        nc.sync.dma_start(transT_sb[:], transT_in[:])
        start_sb = const.tile([128, 1], F32)
        nc.sync.dma_start(start_sb[:], start_in[:])
        stop_sb = const.tile([128, 1], F32)
        nc.sync.dma_start(stop_sb[:], stop_in[:])

        for ti in range(2):
            fT = const.tile([128, Tn], F32, tag=f"fT{ti}")
            for b4 in range(4):
                b = ti * 4 + b4
                nc.sync.dma_start(
                    fT[b4 * 32:(b4 + 1) * 32, :],
                    feats_sb[:, :, b:b + 1].rearrange("p t o -> p (t o)"),
                )
            dpc = const.tile([128, 1], F32, tag=f"dpc{ti}")
            nc.vector.tensor_tensor(out=dpc[:], in0=fT[:, 0:1], in1=start_sb[:], op=OP.add)
            bp_st = const.tile([128, Tn, 8], U32, tag=f"bp{ti}")
            nc.vector.memset(bp_st[:, 0, :], 0)
            for t in range(1, Tn):
                A = temps.tile([128, K], F32, tag=f"A{ti}")
                nc.vector.tensor_tensor(
                    out=A[:], in0=dpc[:, 0:1].to_broadcast((128, K)),
                    in1=transT_sb[:], op=OP.add,
                )
                At = temps.tile([128, K], F32, tag=f"At{ti}")
                nc.vector.transpose(At[:], A[:])
                vm = temps.tile([128, 8], F32, tag=f"vm{ti}")
                nc.vector.max(vm[:], At[:])
                nc.vector.max_index(bp_st[:, t, :], vm[:], At[:])
                nc.vector.tensor_tensor(out=dpc[:], in0=vm[:, 0:1], in1=fT[:, t:t + 1], op=OP.add)
            nc.vector.tensor_tensor(out=dpc[:], in0=dpc[:, 0:1], in1=stop_sb[:], op=OP.add)
            fin = temps.tile([128, 32], F32, tag=f"fin{ti}")
            nc.vector.memset(fin[:], -1e30)
            nc.vector.tensor_copy(fin[:, 0:1], dpc[:])
            finT = temps.tile([128, 32], F32, tag=f"finT{ti}")
            nc.vector.transpose(finT[:], fin[:])
            fm = temps.tile([128, 8], F32, tag=f"fm{ti}")
            nc.vector.max(fm[:], finT[:])
            fi = temps.tile([128, 8], U32, tag=f"fi{ti}")
            nc.vector.max_index(fi[:], fm[:], finT[:])
            nc.sync.dma_start(scores_out[ti], fm[:])
            nc.sync.dma_start(last_out[ti], fi[:])
            nc.sync.dma_start(bp_out[ti], bp_st[:])

    nc.compile()
    return nc


_NC_CACHE = {}


def _get_nc():
    if "nc" not in _NC_CACHE:
        _NC_CACHE["nc"] = build_kernel()
    return _NC_CACHE["nc"]


def _prep_inputs(inputs):
    sent = np.asarray(inputs["sentences"])[:_T]
    emb16 = _f16(inputs["emb"])
    whh = np.stack([_prep_wT(inputs[f"w_hh_l{l}{d}"]) for l in range(2) for d in "fr"])
    wih0 = np.stack([_prep_wT(inputs[f"w_ih_l0{d}"]) for d in "fr"])
    wih1 = np.stack([_prep_wT(inputs[f"w_ih_l1{d}"]) for d in "fr"])
    bias = np.stack([
        (np.asarray(inputs[f"b_ih_l{l}{d}"], np.float32)
         + np.asarray(inputs[f"b_hh_l{l}{d}"], np.float32))[_GIDX].reshape(16, 128).T
        for l in range(2) for d in "fr"
    ], axis=1)  # [128, 4, 16]
    W_out = np.asarray(inputs["W_out"], np.float32)
    woutT = _f16(W_out.T.reshape(8, 128, K).transpose(1, 0, 2))
    bout = np.asarray(inputs["b_out"], np.float32)
    trans = np.asarray(inputs["transitions"], np.float32)
    transT = np.tile((trans + bout[:, None]).T, (4, 1)).astype(np.float32)  # [(4b x 32k), j]
    startc = np.tile(np.asarray(inputs["start_trans"], np.float32) + bout, 4).reshape(128, 1)
    stopc = np.tile(np.asarray(inputs["stop_trans"], np.float32), 4).reshape(128, 1)
    common = {
        "emb": emb16, "whh": whh, "wih0": wih0, "wih1": wih1,
        "bias": np.ascontiguousarray(bias, np.float32), "woutT": woutT,
        "transT": transT, "startc": startc.astype(np.float32),
        "stopc": stopc.astype(np.float32),
    }
    in_maps = []
    for c in range(NCORES):
        toks = np.ascontiguousarray(
            sent[:, c * BC:(c + 1) * BC].reshape(_T * BC, 1).astype(np.int32))
        in_maps.append({**common, "toks": toks})
    return in_maps


def kernel(**inputs):
    from concourse.bass_utils import run_bass_kernel_spmd

    nc = _get_nc()
    in_maps = _prep_inputs(inputs)
    res = run_bass_kernel_spmd(nc, in_maps, list(range(NCORES)))

    scores = np.zeros((B,), np.float32)
    paths = np.zeros((_T, B), np.int32)
    for c in range(NCORES):
        r = res.results[c]
        for ti in range(2):
            bp = r["bp"][ti]  # [128, Tn, 8]
            for b4 in range(4):
                b = c * BC + ti * 4 + b4
                scores[b] = r["scores"][ti, b4 * 32, 0]
                tag = int(r["last"][ti, b4 * 32, 0])
                paths[_T - 1, b] = tag
                for t in range(_T - 1, 0, -1):
                    tag = int(bp[b4 * 32 + tag, t, 0])
                    paths[t - 1, b] = tag
    return scores, paths
